# revision 1
# baseline (speedup 1.0000x reference)
"""Trainium2 Bass kernel for nn_CaptionDecoder (embedding -> masked LSTM -> vocab projection).

Sharding: the LSTM (B=32, S=64, H=512) is replicated on all 8 cores; the
vocab dimension of W_out/b_out is sharded 8-way (4000 per core). Each core
emits logits [S*B, 4000]; the host concatenates along vocab.

Device dataflow per core:
  - gather token embeddings via indirect DMA (128 tokens/tile, t-major order)
  - PE-transpose them to emb_T [e, tok] layout
  - per 4-step group: xg = W_x.T-free matmul into a PSUM block [128=4*32, 2048],
    + bias via a K=1 rank-1 matmul
  - each LSTM step s accumulates h_{t-1} @ W_h INTO rows [32s:32s+32] of that
    same PSUM block (base-partition offset matmul), so z = xg + h@W_h + b is
    materialized with zero extra copies
  - gates: ScalarE sigmoid/tanh straight out of PSUM; state update + Keras
    zero-token masking on VectorE (copy_predicated with a [32,1] mask column)
  - h is PE-transposed back each step into a [128, 8*32] ring that serves as
    the stationary lhsT for both the next steps and the group's logits matmul
  - logits: W_out streamed as the moving operand against the 4-step h block
    (M=128), bias via K=1 matmul, ScalarE copy to SBUF, DMA out.
"""

import sys

import numpy as np

if "/opt/trn_rl_repo" not in sys.path:
    sys.path.insert(0, "/opt/trn_rl_repo")

import concourse.bass as bass
import concourse.bacc as bacc
import concourse.mybir as mybir
import concourse.tile as tile
from concourse.bass_utils import run_bass_kernel_spmd
from concourse.masks import make_identity

VOCAB, EMBED, HIDDEN, CTX = 32000, 512, 512, 2048
B, S = 32, 64
G4 = 4 * HIDDEN  # 2048 gate width
NCORES = 8
VSH = VOCAB // NCORES  # 4000 vocab per core
P = 128
T = S * B  # 2048 tokens, t-major (tok = t*B + b)
NT = T // P  # 16 token tiles / groups
NK = HIDDEN // P  # 4 k-chunks over hidden/embed
NKC = CTX // P  # 16 k-chunks over context
NV = 8  # vocab slices per core
VS = VSH // NV  # 500 wide each
F32 = mybir.dt.float32
BF = mybir.dt.bfloat16
I32 = mybir.dt.int32

_CACHE: dict = {}



def _build_program() -> bass.Bass:
    nc = bacc.Bacc(None)

    ctx_d = nc.declare_dram_parameter("context_t", [CTX, B], BF, isOutput=False)
    embt_d = nc.declare_dram_parameter("emb_t", [EMBED, T], BF, isOutput=False)
    wih_d = nc.declare_dram_parameter("w_ih", [CTX, HIDDEN], BF, isOutput=False)
    wic_d = nc.declare_dram_parameter("w_ic", [CTX, HIDDEN], BF, isOutput=False)
    wx_d = nc.declare_dram_parameter("w_x", [EMBED, G4], BF, isOutput=False)
    wh_d = nc.declare_dram_parameter("w_h", [HIDDEN, G4], BF, isOutput=False)
    bg_d = nc.declare_dram_parameter("b_g", [G4], BF, isOutput=False)
    bih_d = nc.declare_dram_parameter("b_ih", [HIDDEN], BF, isOutput=False)
    bic_d = nc.declare_dram_parameter("b_ic", [HIDDEN], BF, isOutput=False)
    wout_d = nc.declare_dram_parameter("w_out", [HIDDEN, VSH], BF, isOutput=False)
    bout_d = nc.declare_dram_parameter("b_out", [VSH], BF, isOutput=False)
    mask_d = nc.declare_dram_parameter("maskf", [B, S], mybir.dt.uint8, isOutput=False)
    out_d = nc.declare_dram_parameter("logits", [T, VSH], F32, isOutput=True)

    with tile.TileContext(nc) as tc:
        with (
            tc.tile_pool(name="const", bufs=1) as cp,
            tc.tile_pool(name="stream", bufs=2) as sp,
            tc.tile_pool(name="embp", bufs=2) as ep,
            tc.tile_pool(name="state", bufs=1) as st,
            tc.tile_pool(name="gates", bufs=1) as gp,
            tc.tile_pool(name="lout", bufs=2) as lp,
            tc.tile_pool(name="pz", bufs=1, space="PSUM") as pz,
            tc.tile_pool(name="pa", bufs=2, space="PSUM") as pa,
            tc.tile_pool(name="pb", bufs=2, space="PSUM") as pb,
        ):
            # ---- resident constants / weights ----
            ident = cp.tile([P, P], F32, tag="ident", name="ident")
            make_identity(nc, ident[:])
            ones1 = cp.tile([1, P], BF, tag="ones1", name="ones1")
            nc.vector.memset(ones1[:], 1.0)

            ctx_sb = cp.tile([P, NKC * B], BF, tag="ctx", name="ctx")
            nc.sync.dma_start(
                out=ctx_sb[:].rearrange("p (k b) -> p k b", b=B),
                in_=ctx_d.rearrange("(k p) b -> p k b", p=P),
            )
            mask_sb = cp.tile([B, S], mybir.dt.uint8, tag="mask", name="mask")
            nc.sync.dma_start(out=mask_sb[:], in_=mask_d[:, :])
            bg_sb = cp.tile([1, G4], BF, tag="bg", name="bg")
            nc.sync.dma_start(out=bg_sb[:], in_=bg_d[None, :])
            bout_sb = cp.tile([1, VSH], BF, tag="bout", name="bout")
            nc.sync.dma_start(out=bout_sb[:], in_=bout_d[None, :])
            bih_sb = cp.tile([1, HIDDEN], BF, tag="bih", name="bih")
            nc.sync.dma_start(out=bih_sb[:], in_=bih_d[None, :])
            bic_sb = cp.tile([1, HIDDEN], BF, tag="bic", name="bic")
            nc.sync.dma_start(out=bic_sb[:], in_=bic_d[None, :])

            wh_sb = []
            wx_sb = []
            wout_sb = []
            for k in range(NK):
                t_wh = cp.tile([P, G4], BF, tag=f"wh{k}", name=f"wh{k}")
                nc.sync.dma_start(out=t_wh[:], in_=wh_d[k * P : (k + 1) * P, :])
                wh_sb.append(t_wh)
                t_wx = cp.tile([P, G4], BF, tag=f"wx{k}", name=f"wx{k}")
                nc.sync.dma_start(out=t_wx[:], in_=wx_d[k * P : (k + 1) * P, :])
                wx_sb.append(t_wx)
                t_wo = cp.tile([P, VSH], BF, tag=f"wout{k}", name=f"wout{k}")
                nc.sync.dma_start(out=t_wo[:], in_=wout_d[k * P : (k + 1) * P, :])
                wout_sb.append(t_wo)

            # ---- initial state h0/c0 = tanh(context @ W) ----
            # out [b=32, h=512]: lhsT = context_T chunk [128, 32] (stationary),
            # rhs = W_ih chunk [128, 512] streamed from DRAM.
            h_st = [st.tile([B, HIDDEN], F32, tag=f"h{i}", name=f"h{i}") for i in range(2)]
            c_st = [st.tile([B, HIDDEN], F32, tag=f"c{i}", name=f"c{i}") for i in range(2)]
            for w_dram, b_sb, dst in (
                (wih_d, bih_sb, h_st[0]),
                (wic_d, bic_sb, c_st[0]),
            ):
                ps = pb.tile([B, HIDDEN], F32, tag="pbt", name="pbt")
                for kc in range(NKC):
                    wt = sp.tile([P, HIDDEN], BF, tag="wstream", name="wstream")
                    nc.sync.dma_start(out=wt[:], in_=w_dram[kc * P : (kc + 1) * P, :])
                    nc.tensor.matmul(
                        out=ps[:],
                        lhsT=(ctx_sb[:, kc * B : (kc + 1) * B]),
                        rhs=(wt[:]),
                        start=(kc == 0),
                        stop=False,
                    )
                nc.tensor.matmul(
                    out=ps[:],
                    lhsT=(ones1[:1, :B]),
                    rhs=(b_sb[:1, :]),
                    start=False,
                    stop=True,
                )
                nc.scalar.activation(dst[:], ps[:], mybir.ActivationFunctionType.Tanh)

            # h transpose ring: slot(t) = t % 8 holds h_t as [h, b] column block;
            # groups alternate halves so each group's 4 slots form a [128,128] lhsT.
            ring = [cp.tile([P, 8 * B], BF, tag=f"ring{k}", name=f"ring{k}") for k in range(NK)]
            h0T = [cp.tile([P, B], BF, tag=f"h0T{k}", name=f"h0T{k}") for k in range(NK)]

            def transpose_h(src, dests):
                # src [32, 512] -> dests[k][:, col_slice] = src[:, k*128:+128].T
                for k in range(NK):
                    tp = pb.tile([P, B], F32, tag="pbt", name="pbt")
                    nc.tensor.transpose(
                        out=tp[:],
                        in_=src[:, k * P : (k + 1) * P],
                        identity=ident[:B, :B],
                    )
                    nc.vector.tensor_copy(dests[k], tp[:])

            transpose_h(h_st[0][:], [h0T[k][:, :] for k in range(NK)])

            # ---- pre-gathered, pre-transposed embeddings streamed per group ----
            def load_embT(g):
                ts = []
                for k in range(NK):
                    et = ep.tile([P, P], BF, tag=f"embT{k}", name=f"embT{k}")
                    nc.sync.dma_start(
                        out=et[:],
                        in_=embt_d[k * P : (k + 1) * P, g * P : (g + 1) * P],
                    )
                    ts.append(et)
                return ts

            embT_cur = load_embT(0)

            sig = mybir.ActivationFunctionType.Sigmoid
            tanh = mybir.ActivationFunctionType.Tanh

            for g in range(NT):
                # prefetch next group's embeddings
                embT_nxt = None
                if g + 1 < NT:
                    embT_nxt = load_embT(g + 1)

                # xg for this group into the shared PSUM block [128, 2048]
                xz = pz.tile([P, G4], F32, tag="xz", name="xz")
                for n in range(4):
                    ns = slice(n * HIDDEN, (n + 1) * HIDDEN)
                    for k in range(NK):
                        nc.tensor.matmul(
                            out=xz[:, ns],
                            lhsT=(embT_cur[k][:]),
                            rhs=(wx_sb[k][:, ns]),
                            start=(k == 0),
                            stop=False,
                        )
                    nc.tensor.matmul(
                        out=xz[:, ns],
                        lhsT=(ones1[:1, :]),
                        rhs=(bg_sb[:1, ns]),
                        start=False,
                        stop=True,
                    )

                # ---- 4 LSTM steps accumulating into rows of xz ----
                for s in range(4):
                    t = 4 * g + s
                    rows = slice(B * s, B * (s + 1))
                    if t == 0:
                        hT_prev = [h0T[k][:, :] for k in range(NK)]
                    else:
                        sl = ((t - 1) % 8) * B
                        hT_prev = [ring[k][:, sl : sl + B] for k in range(NK)]

                    for n in range(4):
                        ns = slice(n * HIDDEN, (n + 1) * HIDDEN)
                        for k in range(NK):
                            nc.tensor.matmul(
                                out=xz[rows, ns],
                                lhsT=(hT_prev[k]),
                                rhs=(wh_sb[k][:, ns]),
                                start=False,
                                stop=False,
                                tile_position=(0, B * s),
                                skip_group_check=True,
                            )

                    # gates from PSUM rows (Keras order i, f, g, o)
                    sig_i = gp.tile([B, HIDDEN], F32, tag="sig_i", name="sig_i")
                    sig_f = gp.tile([B, HIDDEN], F32, tag="sig_f", name="sig_f")
                    tanh_g = gp.tile([B, HIDDEN], F32, tag="tanh_g", name="tanh_g")
                    sig_o = gp.tile([B, HIDDEN], F32, tag="sig_o", name="sig_o")
                    nc.scalar.activation(sig_i[:], xz[rows, 0:HIDDEN], sig)
                    nc.scalar.activation(sig_f[:], xz[rows, HIDDEN : 2 * HIDDEN], sig)
                    nc.scalar.activation(
                        tanh_g[:], xz[rows, 2 * HIDDEN : 3 * HIDDEN], tanh
                    )
                    nc.scalar.activation(sig_o[:], xz[rows, 3 * HIDDEN : 4 * HIDDEN], sig)

                    h_prev = h_st[t % 2]
                    c_prev = c_st[t % 2]
                    h_next = h_st[(t + 1) % 2]
                    c_next = c_st[(t + 1) % 2]

                    c_new = gp.tile([B, HIDDEN], F32, tag="c_new", name="c_new")
                    tmp = gp.tile([B, HIDDEN], F32, tag="tmp", name="tmp")
                    nc.vector.tensor_mul(c_new[:], sig_f[:], c_prev[:])
                    nc.vector.tensor_mul(tmp[:], sig_i[:], tanh_g[:])
                    nc.vector.tensor_add(c_new[:], c_new[:], tmp[:])

                    tanh_c = gp.tile([B, HIDDEN], F32, tag="tanh_c", name="tanh_c")
                    nc.scalar.activation(tanh_c[:], c_new[:], tanh)
                    h_new = gp.tile([B, HIDDEN], F32, tag="h_new", name="h_new")
                    nc.vector.tensor_mul(h_new[:], sig_o[:], tanh_c[:])

                    # Keras masking: masked (token==0) steps carry prev state
                    m_bc = mask_sb[:, t : t + 1].to_broadcast([B, HIDDEN])
                    nc.vector.tensor_copy(c_next[:], c_prev[:])
                    nc.vector.copy_predicated(c_next[:], m_bc, c_new[:])
                    nc.vector.tensor_copy(h_next[:], h_prev[:])
                    nc.vector.copy_predicated(h_next[:], m_bc, h_new[:])

                    sl = (t % 8) * B
                    transpose_h(
                        h_next[:], [ring[k][:, sl : sl + B] for k in range(NK)]
                    )

                # ---- logits for this group: [128 tokens, VSH] ----
                half = (g % 2) * (4 * B)
                for v in range(NV):
                    vs = slice(v * VS, (v + 1) * VS)
                    pl = pa.tile([P, VS], F32, tag="pa", name="pa")
                    for k in range(NK):
                        nc.tensor.matmul(
                            out=pl[:],
                            lhsT=(ring[k][:, half : half + 4 * B]),
                            rhs=(wout_sb[k][:, vs]),
                            start=(k == 0),
                            stop=False,
                        )
                    nc.tensor.matmul(
                        out=pl[:],
                        lhsT=(ones1[:1, :]),
                        rhs=(bout_sb[:1, vs]),
                        start=False,
                        stop=True,
                    )
                    lo = lp.tile([P, VS], F32, tag="lo", name="lo")
                    nc.scalar.copy(lo[:], pl[:])
                    nc.sync.dma_start(
                        out=out_d[g * P : (g + 1) * P, vs], in_=lo[:]
                    )

                embT_cur = embT_nxt

    return nc


def _get_program() -> bass.Bass:
    if "nc" not in _CACHE:
        _CACHE["nc"] = _build_program()
    return _CACHE["nc"]


def prep_in_maps(inputs) -> list:
    import ml_dtypes

    bf16 = ml_dtypes.bfloat16
    tok = np.asarray(inputs["target_tokens"])
    ctx = np.asarray(inputs["context"], dtype=np.float32)
    emb_table = np.asarray(inputs["emb_table"], np.float32)
    w_out = np.asarray(inputs["W_out"], np.float32)
    b_out = np.asarray(inputs["b_out"], np.float32)

    mask = (tok != 0).astype(np.uint8)  # [B, S]
    tok_t = tok.T.reshape(-1).astype(np.int64)  # t*B + b token order
    emb_t = np.ascontiguousarray(emb_table[tok_t].T.astype(bf16))  # [EMBED, T]
    ctx_t = np.ascontiguousarray(ctx.T.astype(bf16))  # [CTX, B]

    shared = {
        "context_t": ctx_t,
        "emb_t": emb_t,
        "w_ih": np.ascontiguousarray(np.asarray(inputs["W_ih"]).astype(bf16)),
        "w_ic": np.ascontiguousarray(np.asarray(inputs["W_ic"]).astype(bf16)),
        "w_x": np.ascontiguousarray(np.asarray(inputs["W_x"]).astype(bf16)),
        "w_h": np.ascontiguousarray(np.asarray(inputs["W_h"]).astype(bf16)),
        "b_g": np.ascontiguousarray(np.asarray(inputs["b"]).astype(bf16)),
        "b_ih": np.ascontiguousarray(np.asarray(inputs["b_ih"]).astype(bf16)),
        "b_ic": np.ascontiguousarray(np.asarray(inputs["b_ic"]).astype(bf16)),
        "maskf": np.ascontiguousarray(mask),
    }
    in_maps = []
    for j in range(NCORES):
        m = dict(shared)
        m["w_out"] = np.ascontiguousarray(w_out[:, j * VSH : (j + 1) * VSH].astype(bf16))
        m["b_out"] = np.ascontiguousarray(b_out[j * VSH : (j + 1) * VSH].astype(bf16))
        in_maps.append(m)
    return in_maps


def kernel(**inputs: np.ndarray) -> np.ndarray:
    in_maps = prep_in_maps(inputs)
    nc = _get_program()
    if not nc.is_finalized():
        nc.finalize()

    import os

    trace = bool(os.environ.get("CAPDEC_TRACE"))
    kw = {}
    if trace:
        kw["trace"] = True
        tdir = os.environ.get("CAPDEC_TRACE_DIR")
        if tdir:
            os.makedirs(tdir, exist_ok=True)
            kw["tmpdir"] = tdir
    bkr = run_bass_kernel_spmd(nc, in_maps, list(range(NCORES)), **kw)
    _CACHE["last_results"] = bkr
    res = bkr.results
    parts = [res[j]["logits"].reshape(S, B, VSH) for j in range(NCORES)]
    full = np.concatenate(parts, axis=-1)  # [S, B, VOCAB]
    return np.ascontiguousarray(full.transpose(1, 0, 2))



# revision 4
# speedup vs baseline: 1.2411x; 1.2411x over previous
"""Trainium2 Bass kernel for nn_CaptionDecoder (embedding -> masked LSTM -> vocab projection).

Sharding: the LSTM (B=32, S=64, H=512) is replicated on all 8 cores; the
vocab dimension of W_out/b_out is sharded 8-way (4000 per core). Each core
emits logits [S*B, 4000] in bf16; the host concatenates along vocab and
converts to f32.

Device dataflow per core (pipelined across 16 groups of 4 LSTM steps):
  - emb gathered+transposed on host -> emb_t [E, T] in DRAM, streamed per group
  - xg = emb@W_x + b staged into SBUF (bf16) one group ahead through a small
    PSUM buffer; injected into the group's big PSUM block [128, 2048] with a
    single identity matmul (4 instrs), so the PE can run this + the previous
    group's logits as filler work inside the recurrence's latency gaps
  - each LSTM step s: h_{t-1} @ W_h accumulated into rows [32s:32s+32] of the
    PSUM block (tile_position col offset); ScalarE activations (bf16 out),
    DVE state update in place with copy_predicated masking (Keras mask_zero)
  - h is PE-transposed (4x into one PSUM tile) and scatter-copied into an
    8-slot ring that serves as lhsT for both the next steps' matmuls and the
    group's logits matmuls
  - logits: ring block [128,128] stationary, W_out streamed; bias added during
    the PSUM->SBUF copy (GpSimd tensor_add vs a row-replicated bias tile);
    DMA out as bf16.
"""

import sys

import numpy as np

if "/opt/trn_rl_repo" not in sys.path:
    sys.path.insert(0, "/opt/trn_rl_repo")

import concourse.bass as bass
import concourse.bacc as bacc
import concourse.mybir as mybir
import concourse.tile as tile
from concourse.bass_utils import run_bass_kernel_spmd
from concourse.masks import make_identity

VOCAB, EMBED, HIDDEN, CTX = 32000, 512, 512, 2048
B, S = 32, 64
G4 = 4 * HIDDEN  # 2048 gate width
NCORES = 8
VSH = VOCAB // NCORES  # 4000 vocab per core
P = 128
T = S * B  # 2048 tokens, t-major (tok = t*B + b)
NT = T // P  # 16 token tiles / groups
NK = HIDDEN // P  # 4 k-chunks over hidden/embed
NKC = CTX // P  # 16 k-chunks over context
NV = 8  # vocab slices per core
VS = VSH // NV  # 500 wide each
F32 = mybir.dt.float32
BF = mybir.dt.bfloat16
I32 = mybir.dt.int32

_CACHE: dict = {}

sig = mybir.ActivationFunctionType.Sigmoid
tanh = mybir.ActivationFunctionType.Tanh


def _build_program() -> bass.Bass:
    nc = bacc.Bacc(None)

    ctx_d = nc.declare_dram_parameter("context_t", [CTX, B], BF, isOutput=False)
    embt_d = nc.declare_dram_parameter("emb_t", [EMBED, T], BF, isOutput=False)
    wih_d = nc.declare_dram_parameter("w_ih", [CTX, HIDDEN], BF, isOutput=False)
    wic_d = nc.declare_dram_parameter("w_ic", [CTX, HIDDEN], BF, isOutput=False)
    wx_d = nc.declare_dram_parameter("w_x", [EMBED, G4], BF, isOutput=False)
    wh_d = nc.declare_dram_parameter("w_h", [HIDDEN, G4], BF, isOutput=False)
    bg_d = nc.declare_dram_parameter("b_g", [G4], BF, isOutput=False)
    bih_d = nc.declare_dram_parameter("b_ih", [HIDDEN], BF, isOutput=False)
    bic_d = nc.declare_dram_parameter("b_ic", [HIDDEN], BF, isOutput=False)
    wout_d = nc.declare_dram_parameter("w_out", [HIDDEN, VSH], BF, isOutput=False)
    bout_d = nc.declare_dram_parameter("b_out", [VSH], BF, isOutput=False)
    mask_d = nc.declare_dram_parameter("maskf", [B, S], mybir.dt.uint8, isOutput=False)
    out_d = nc.declare_dram_parameter("logits", [T, VSH], BF, isOutput=True)

    with tile.TileContext(nc) as tc:
        with (
            tc.tile_pool(name="const", bufs=1) as cp,
            tc.tile_pool(name="stream", bufs=2) as sp,
            tc.tile_pool(name="embp", bufs=2) as ep,
            tc.tile_pool(name="xgp", bufs=2) as xp,
            tc.tile_pool(name="gates", bufs=2) as gp,
            tc.tile_pool(name="lout", bufs=3) as lp,
            tc.tile_pool(name="pz", bufs=1, space="PSUM") as pz,
            tc.tile_pool(name="pstage", bufs=1, space="PSUM") as psg,
            tc.tile_pool(name="pa", bufs=2, space="PSUM") as pa,
            tc.tile_pool(name="ptr", bufs=1, space="PSUM") as pt,
        ):
            # ---- resident constants / weights ----
            identb = cp.tile([P, P], BF, tag="identb", name="identb")
            make_identity(nc, identb[:])
            ones1 = cp.tile([1, P], BF, tag="ones1", name="ones1")
            nc.vector.memset(ones1[:], 1.0)

            ctx_sb = cp.tile([P, NKC * B], BF, tag="ctx", name="ctx")
            nc.sync.dma_start(
                out=ctx_sb[:].rearrange("p (k b) -> p k b", b=B),
                in_=ctx_d.rearrange("(k p) b -> p k b", p=P),
            )
            mask_sb = cp.tile([B, S], mybir.dt.uint8, tag="mask", name="mask")
            nc.sync.dma_start(out=mask_sb[:], in_=mask_d[:, :])
            bg_sb = cp.tile([1, G4], BF, tag="bg", name="bg")
            nc.sync.dma_start(out=bg_sb[:], in_=bg_d[None, :])
            bout_sb = cp.tile([1, VSH], BF, tag="bout", name="bout")
            nc.sync.dma_start(out=bout_sb[:], in_=bout_d[None, :])
            bih_sb = cp.tile([1, HIDDEN], BF, tag="bih", name="bih")
            nc.sync.dma_start(out=bih_sb[:], in_=bih_d[None, :])
            bic_sb = cp.tile([1, HIDDEN], BF, tag="bic", name="bic")
            nc.sync.dma_start(out=bic_sb[:], in_=bic_d[None, :])

            wh_sb = []
            wx_sb = []
            wout_sb = []
            for k in range(NK):
                t_wh = cp.tile([P, G4], BF, tag=f"wh{k}", name=f"wh{k}")
                nc.sync.dma_start(out=t_wh[:], in_=wh_d[k * P : (k + 1) * P, :])
                wh_sb.append(t_wh)
                t_wx = cp.tile([P, G4], BF, tag=f"wx{k}", name=f"wx{k}")
                nc.sync.dma_start(out=t_wx[:], in_=wx_d[k * P : (k + 1) * P, :])
                wx_sb.append(t_wx)
                t_wo = cp.tile([P, VSH], BF, tag=f"wout{k}", name=f"wout{k}")
                nc.sync.dma_start(out=t_wo[:], in_=wout_d[k * P : (k + 1) * P, :])
                wout_sb.append(t_wo)

            # row-replicated b_out for fused bias-add during logits copy
            bout_rep = cp.tile([P, VSH], BF, tag="boutrep", name="boutrep")
            for v in range(NV):
                vs = slice(v * VS, (v + 1) * VS)
                pbo = pa.tile([P, VS], F32, tag="pl", name="pbo")
                nc.tensor.matmul(
                    out=pbo[:],
                    lhsT=(ones1[:1, :]),
                    rhs=(bout_sb[:1, vs]),
                    start=True,
                    stop=True,
                )
                nc.scalar.copy(bout_rep[:, vs], pbo[:])

            # ---- embedding tiles (prefetched), staged xg in SBUF ----
            def load_embT(g):
                ts = []
                for k in range(NK):
                    et = ep.tile([P, P], BF, tag=f"embT{k}", name=f"embT{k}")
                    nc.sync.dma_start(
                        out=et[:],
                        in_=embt_d[k * P : (k + 1) * P, g * P : (g + 1) * P],
                    )
                    ts.append(et)
                return ts

            def stage_xg(embT):
                """xg = emb @ W_x + b for one group -> SBUF bf16 [128, 2048]."""
                xg = xp.tile([P, G4], BF, tag="xg", name="xg")
                for n in range(4):
                    ns = slice(n * HIDDEN, (n + 1) * HIDDEN)
                    ps_t = psg.tile([P, HIDDEN], F32, tag="xs", name="ps_t")
                    for k in range(NK):
                        nc.tensor.matmul(
                            out=ps_t[:],
                            lhsT=(embT[k][:]),
                            rhs=(wx_sb[k][:, ns]),
                            start=(k == 0),
                            stop=False,
                        )
                    nc.tensor.matmul(
                        out=ps_t[:],
                        lhsT=(ones1[:1, :]),
                        rhs=(bg_sb[:1, ns]),
                        start=False,
                        stop=True,
                    )
                    nc.any.tensor_copy(xg[:, ns], ps_t[:])
                return xg

            # ---- initial state h0/c0 = tanh(context @ W + b), using pz block ----
            h_sb = cp.tile([B, HIDDEN], BF, tag="h", name="h")
            c_sb = cp.tile([B, HIDDEN], F32, tag="c", name="c")

            # h transpose ring: slot(t) = t % 8, cols (k*8 + slot)*32
            ring = cp.tile([P, NK * 8 * B], BF, tag="ring", name="ring")

            def transpose_h(t):
                """PE-transpose h [32,512] into ring slot t%8 via one psum tile."""
                slot = t % 8
                tp = pt.tile([P, P], BF, tag="tp", name="tp")
                for k in range(NK):
                    nc.tensor.transpose(
                        out=tp[:, k * B : (k + 1) * B],
                        in_=h_sb[:, k * P : (k + 1) * P],
                        identity=identb[:B, :B],
                    )
                dst = ring[:].rearrange("p (k s c) -> p k s c", k=NK, s=8)[
                    :, :, slot, :
                ]
                src = tp[:].rearrange("p (k c) -> p k c", k=NK)
                nc.vector.tensor_copy(dst, src)

            embT_cur = load_embT(0)
            embT_nxt = load_embT(1)

            xz0 = pz.tile([P, G4], F32, tag="xz", name="xz0")
            for idx, (w_dram, b_sb) in enumerate(
                ((wih_d, bih_sb), (wic_d, bic_sb))
            ):
                reg = slice(idx * HIDDEN, (idx + 1) * HIDDEN)
                for kc in range(NKC):
                    wt = sp.tile([P, HIDDEN], BF, tag="wstream", name="wstream")
                    nc.sync.dma_start(out=wt[:], in_=w_dram[kc * P : (kc + 1) * P, :])
                    nc.tensor.matmul(
                        out=xz0[:B, reg],
                        lhsT=(ctx_sb[:, kc * B : (kc + 1) * B]),
                        rhs=(wt[:]),
                        start=(kc == 0),
                        stop=False,
                    )
                nc.tensor.matmul(
                    out=xz0[:B, reg],
                    lhsT=(ones1[:1, :B]),
                    rhs=(b_sb[:1, :]),
                    start=False,
                    stop=True,
                )
            nc.scalar.activation(h_sb[:], xz0[:B, 0:HIDDEN], tanh)
            nc.scalar.activation(c_sb[:], xz0[:B, HIDDEN : 2 * HIDDEN], tanh)
            transpose_h(-1)  # h0 into slot 7

            xg_cur = stage_xg(embT_cur)

            def logits_group(g):
                """Vocab-sharded logits for token tile g from ring slots."""
                half = (g % 2) * 4
                for v in range(NV):
                    vs = slice(v * VS, (v + 1) * VS)
                    pl = pa.tile([P, VS], F32, tag="pl", name="pl")
                    for k in range(NK):
                        cbase = (k * 8 + half) * B
                        nc.tensor.matmul(
                            out=pl[:],
                            lhsT=(ring[:, cbase : cbase + 4 * B]),
                            rhs=(wout_sb[k][:, vs]),
                            start=(k == 0),
                            stop=(k == NK - 1),
                        )
                    lo = lp.tile([P, VS], BF, tag="lo", name="lo")
                    nc.any.tensor_add(lo[:], pl[:], bout_rep[:, vs])
                    nc.sync.dma_start(out=out_d[g * P : (g + 1) * P, vs], in_=lo[:])

            # ---- main loop ----
            for g in range(NT):
                # inject staged xg into the big PSUM block (resets accumulation)
                xz = pz.tile([P, G4], F32, tag="xz", name="xz")
                for n in range(4):
                    ns = slice(n * HIDDEN, (n + 1) * HIDDEN)
                    nc.tensor.matmul(
                        out=xz[:, ns],
                        lhsT=(identb[:]),
                        rhs=(xg_cur[:, ns]),
                        start=True,
                        stop=True,
                    )

                for s in range(4):
                    t = 4 * g + s
                    rows = slice(B * s, B * (s + 1))
                    slot_prev = (t - 1) % 8

                    # recurrent part: z[rows] += h_{t-1} @ W_h (gate-major f,i,g,o)
                    for n in (1, 0, 2, 3):  # f, i, g, o (Keras order i,f,g,o)
                        ns = slice(n * HIDDEN, (n + 1) * HIDDEN)
                        for k in range(NK):
                            cbase = (k * 8 + slot_prev) * B
                            nc.tensor.matmul(
                                out=xz[rows, ns],
                                lhsT=(ring[:, cbase : cbase + B]),
                                rhs=(wh_sb[k][:, ns]),
                                start=False,
                                stop=False,
                                tile_position=(0, B * s),
                                skip_group_check=True,
                            )

                    # gates (Keras order i, f, g, o); bf16 outputs
                    sig_f = gp.tile([B, HIDDEN], BF, tag="sig_f", name="sig_f")
                    sig_i = gp.tile([B, HIDDEN], BF, tag="sig_i", name="sig_i")
                    tanh_g = gp.tile([B, HIDDEN], BF, tag="tanh_g", name="tanh_g")
                    sig_o = gp.tile([B, HIDDEN], BF, tag="sig_o", name="sig_o")
                    nc.scalar.activation(sig_f[:], xz[rows, HIDDEN : 2 * HIDDEN], sig)
                    nc.scalar.activation(sig_i[:], xz[rows, 0:HIDDEN], sig)
                    nc.scalar.activation(
                        tanh_g[:], xz[rows, 2 * HIDDEN : 3 * HIDDEN], tanh
                    )
                    nc.scalar.activation(sig_o[:], xz[rows, 3 * HIDDEN : 4 * HIDDEN], sig)

                    t1 = gp.tile([B, HIDDEN], F32, tag="t1", name="t1")
                    t2 = gp.tile([B, HIDDEN], F32, tag="t2", name="t2")
                    c_new = gp.tile([B, HIDDEN], F32, tag="c_new", name="c_new")
                    nc.vector.tensor_mul(t1[:], sig_f[:], c_sb[:])
                    nc.vector.tensor_mul(t2[:], sig_i[:], tanh_g[:])
                    nc.vector.tensor_add(c_new[:], t1[:], t2[:])

                    m_bc = mask_sb[:, t : t + 1].to_broadcast([B, HIDDEN])
                    # masked (token==0) steps carry previous state; in-place blend
                    nc.vector.copy_predicated(c_sb[:], m_bc, c_new[:])

                    # h path uses pre-mask c_new: masked rows discard h_new anyway
                    tanh_c = gp.tile([B, HIDDEN], BF, tag="tanh_c", name="tanh_c")
                    nc.scalar.activation(tanh_c[:], c_new[:], tanh)
                    h_new = gp.tile([B, HIDDEN], BF, tag="h_new", name="h_new")
                    nc.vector.tensor_mul(h_new[:], sig_o[:], tanh_c[:])
                    nc.vector.copy_predicated(h_sb[:], m_bc, h_new[:])

                    transpose_h(t)

                # filler work: stage next group's xg, then previous group's logits
                if g + 1 < NT:
                    xg_cur = stage_xg(embT_nxt)
                    if g + 2 < NT:
                        embT_nxt = load_embT(g + 2)
                if g >= 1:
                    logits_group(g - 1)

            logits_group(NT - 1)

    return nc


def _get_program() -> bass.Bass:
    if "nc" not in _CACHE:
        _CACHE["nc"] = _build_program()
    return _CACHE["nc"]


def prep_in_maps(inputs) -> list:
    import ml_dtypes

    bf16 = ml_dtypes.bfloat16
    tok = np.asarray(inputs["target_tokens"])
    ctx = np.asarray(inputs["context"], dtype=np.float32)
    emb_table = np.asarray(inputs["emb_table"], np.float32)
    w_out = np.asarray(inputs["W_out"], np.float32)
    b_out = np.asarray(inputs["b_out"], np.float32)

    mask = (tok != 0).astype(np.uint8)  # [B, S]
    tok_t = tok.T.reshape(-1).astype(np.int64)  # t*B + b token order
    emb_t = np.ascontiguousarray(emb_table[tok_t].T.astype(bf16))  # [EMBED, T]
    ctx_t = np.ascontiguousarray(ctx.T.astype(bf16))  # [CTX, B]

    shared = {
        "context_t": ctx_t,
        "emb_t": emb_t,
        "w_ih": np.ascontiguousarray(np.asarray(inputs["W_ih"]).astype(bf16)),
        "w_ic": np.ascontiguousarray(np.asarray(inputs["W_ic"]).astype(bf16)),
        "w_x": np.ascontiguousarray(np.asarray(inputs["W_x"]).astype(bf16)),
        "w_h": np.ascontiguousarray(np.asarray(inputs["W_h"]).astype(bf16)),
        "b_g": np.ascontiguousarray(np.asarray(inputs["b"]).astype(bf16)),
        "b_ih": np.ascontiguousarray(np.asarray(inputs["b_ih"]).astype(bf16)),
        "b_ic": np.ascontiguousarray(np.asarray(inputs["b_ic"]).astype(bf16)),
        "maskf": np.ascontiguousarray(mask),
    }
    in_maps = []
    for j in range(NCORES):
        m = dict(shared)
        m["w_out"] = np.ascontiguousarray(w_out[:, j * VSH : (j + 1) * VSH].astype(bf16))
        m["b_out"] = np.ascontiguousarray(b_out[j * VSH : (j + 1) * VSH].astype(bf16))
        in_maps.append(m)
    return in_maps


def kernel(**inputs: np.ndarray) -> np.ndarray:
    in_maps = prep_in_maps(inputs)
    nc = _get_program()
    if not nc.is_finalized():
        nc.finalize()

    import os

    trace = bool(os.environ.get("CAPDEC_TRACE"))
    kw = {}
    if trace:
        kw["trace"] = True
        tdir = os.environ.get("CAPDEC_TRACE_DIR")
        if tdir:
            os.makedirs(tdir, exist_ok=True)
            kw["tmpdir"] = tdir
    bkr = run_bass_kernel_spmd(nc, in_maps, list(range(NCORES)), **kw)
    _CACHE["last_results"] = bkr
    res = bkr.results
    parts = [
        np.asarray(res[j]["logits"]).astype(np.float32).reshape(S, B, VSH)
        for j in range(NCORES)
    ]
    full = np.concatenate(parts, axis=-1)  # [S, B, VOCAB]
    return np.ascontiguousarray(full.transpose(1, 0, 2))


# revision 9
# speedup vs baseline: 1.8389x; 1.4817x over previous
"""Trainium2 Bass kernel for nn_CaptionDecoder (embedding -> masked LSTM -> vocab projection).

Sharding: the LSTM (B=32, S=64, H=512) is replicated on all 8 cores; the
vocab dimension of W_out/b_out is sharded 8-way (4000 per core). Each core
emits logits [S*B, 4000] f32; the host concatenates along vocab.

Device dataflow per core (pipelined across 16 groups of 4 LSTM steps):
  - emb gathered+transposed on host -> emb_t [E, T] in DRAM, streamed per group
  - xg = emb@W_x + b staged into SBUF (bf16) one group ahead through a small
    PSUM buffer; injected into four per-gate PSUM tiles [128, 512] with one
    identity matmul each (per-gate tiles keep activation deps fine-grained)
  - each LSTM step s: h_{t-1} @ W_h accumulated into rows [32s:32s+32] of the
    per-gate PSUM tiles (tile_position col offset); ScalarE activations (bf16),
    DVE state update (bf16 c/h) in place with copy_predicated masking
  - h is PE-transposed (4x into one PSUM tile) and scatter-copied into an
    8-slot ring that serves as lhsT for the next steps and the logits matmuls
  - logits: ring block [128,128] stationary, W_out streamed, bias via K=1
    ones matmul, then DMA'd straight from PSUM to DRAM in f32
  - all filler work (xg staging, logits) is emitted at very low scheduler
    priority so it drips into PE idle gaps instead of delaying the recurrence.
"""

import sys
from contextlib import contextmanager

import numpy as np

if "/opt/trn_rl_repo" not in sys.path:
    sys.path.insert(0, "/opt/trn_rl_repo")

import concourse.bass as bass
import concourse.bacc as bacc
import concourse.mybir as mybir
import concourse.tile as tile
from concourse.bass_utils import run_bass_kernel_spmd
from concourse.masks import make_identity

VOCAB, EMBED, HIDDEN, CTX = 32000, 512, 512, 2048
B, S = 32, 64
G4 = 4 * HIDDEN  # 2048 gate width
NCORES = 8
VSH = VOCAB // NCORES  # 4000 vocab per core
P = 128
T = S * B  # 2048 tokens, t-major (tok = t*B + b)
NT = T // P  # 16 token tiles / groups
NK = HIDDEN // P  # 4 k-chunks over hidden/embed
NKC = CTX // P  # 16 k-chunks over context
NV = 8  # vocab slices per core
VS = VSH // NV  # 500 wide each
F32 = mybir.dt.float32
BF = mybir.dt.bfloat16

_CACHE: dict = {}

sig = mybir.ActivationFunctionType.Sigmoid
tanh = mybir.ActivationFunctionType.Tanh


@contextmanager
def low_priority(tc, bump=1_000_000):
    """Emit instructions as if issued much later: the scheduler only picks
    them when nothing chain-critical is ready (pure filler work)."""
    p = tc.cur_priority
    tc.cur_priority = p + bump
    try:
        yield
    finally:
        tc.cur_priority = p


def _build_program() -> bass.Bass:
    nc = bacc.Bacc(None)

    ctx_d = nc.declare_dram_parameter("context_t", [CTX, B], BF, isOutput=False)
    embt_d = nc.declare_dram_parameter("emb_t", [EMBED, T], BF, isOutput=False)
    wih_d = nc.declare_dram_parameter("w_ih", [CTX, HIDDEN], BF, isOutput=False)
    wic_d = nc.declare_dram_parameter("w_ic", [CTX, HIDDEN], BF, isOutput=False)
    wx_d = nc.declare_dram_parameter("w_x", [EMBED, G4], BF, isOutput=False)
    wh_d = nc.declare_dram_parameter("w_h", [HIDDEN, G4], BF, isOutput=False)
    bg_d = nc.declare_dram_parameter("b_g", [G4], BF, isOutput=False)
    bih_d = nc.declare_dram_parameter("b_ih", [HIDDEN], BF, isOutput=False)
    bic_d = nc.declare_dram_parameter("b_ic", [HIDDEN], BF, isOutput=False)
    wout_d = nc.declare_dram_parameter("w_out", [HIDDEN, VSH], BF, isOutput=False)
    bout_d = nc.declare_dram_parameter("b_out", [VSH], BF, isOutput=False)
    mask_d = nc.declare_dram_parameter("maskf", [B, S], mybir.dt.uint8, isOutput=False)
    out_d = nc.declare_dram_parameter("logits", [T, VSH], BF, isOutput=True)

    with tile.TileContext(nc) as tc:
        with (
            tc.tile_pool(name="const", bufs=1) as cp,
            tc.tile_pool(name="stream", bufs=2) as sp,
            tc.tile_pool(name="embp", bufs=2) as ep,
            tc.tile_pool(name="xgp", bufs=2) as xp,
            tc.tile_pool(name="gates", bufs=2) as gp,
            tc.tile_pool(name="lout", bufs=3) as lp,
            tc.tile_pool(name="pz", bufs=1, space="PSUM") as pz,
            tc.tile_pool(name="pstage", bufs=1, space="PSUM") as psg,
            tc.tile_pool(name="pa", bufs=2, space="PSUM") as pa,
            tc.tile_pool(name="ptr", bufs=1, space="PSUM") as pt,
        ):
            # ---- resident constants / weights ----
            identb = cp.tile([P, P], BF, tag="identb", name="identb")
            make_identity(nc, identb[:])
            ones1 = cp.tile([1, P], BF, tag="ones1", name="ones1")
            nc.vector.memset(ones1[:], 1.0)

            ctx_sb = cp.tile([P, NKC * B], BF, tag="ctx", name="ctx")
            nc.sync.dma_start(
                out=ctx_sb[:].rearrange("p (k b) -> p k b", b=B),
                in_=ctx_d.rearrange("(k p) b -> p k b", p=P),
            )
            mask_sb = cp.tile([B, S], mybir.dt.uint8, tag="mask", name="mask")
            nc.sync.dma_start(out=mask_sb[:], in_=mask_d[:, :])
            bg_sb = cp.tile([1, G4], BF, tag="bg", name="bg")
            nc.sync.dma_start(out=bg_sb[:], in_=bg_d[None, :])
            bout_sb = cp.tile([1, VSH], BF, tag="bout", name="bout")
            nc.sync.dma_start(out=bout_sb[:], in_=bout_d[None, :])
            bih_sb = cp.tile([1, HIDDEN], BF, tag="bih", name="bih")
            nc.sync.dma_start(out=bih_sb[:], in_=bih_d[None, :])
            bic_sb = cp.tile([1, HIDDEN], BF, tag="bic", name="bic")
            nc.sync.dma_start(out=bic_sb[:], in_=bic_d[None, :])

            wh_sb = []
            wx_sb = []
            wout_sb = []
            for k in range(NK):
                t_wh = cp.tile([P, G4], BF, tag=f"wh{k}", name=f"wh{k}")
                nc.sync.dma_start(out=t_wh[:], in_=wh_d[k * P : (k + 1) * P, :])
                wh_sb.append(t_wh)
                t_wx = cp.tile([P, G4], BF, tag=f"wx{k}", name=f"wx{k}")
                nc.sync.dma_start(out=t_wx[:], in_=wx_d[k * P : (k + 1) * P, :])
                wx_sb.append(t_wx)
                t_wo = cp.tile([P, VSH], BF, tag=f"wout{k}", name=f"wout{k}")
                nc.sync.dma_start(out=t_wo[:], in_=wout_d[k * P : (k + 1) * P, :])
                wout_sb.append(t_wo)

            # ---- embedding tiles (prefetched), staged xg in SBUF ----
            def load_embT(g):
                ts = []
                for k in range(NK):
                    et = ep.tile([P, P], BF, tag=f"embT{k}", name=f"embT{k}")
                    nc.sync.dma_start(
                        out=et[:],
                        in_=embt_d[k * P : (k + 1) * P, g * P : (g + 1) * P],
                    )
                    ts.append(et)
                return ts

            def stage_xg(embT):
                """xg = emb @ W_x + b for one group -> SBUF bf16 [128, 2048]."""
                xg = xp.tile([P, G4], BF, tag="xg", name="xg")
                for n in range(4):
                    ns = slice(n * HIDDEN, (n + 1) * HIDDEN)
                    ps_t = psg.tile([P, HIDDEN], F32, tag="xs", name="ps_t")
                    for k in range(NK):
                        nc.tensor.matmul(
                            out=ps_t[:],
                            lhsT=(embT[k][:]),
                            rhs=(wx_sb[k][:, ns]),
                            start=(k == 0),
                            stop=False,
                        )
                    nc.tensor.matmul(
                        out=ps_t[:],
                        lhsT=(ones1[:1, :]),
                        rhs=(bg_sb[:1, ns]),
                        start=False,
                        stop=True,
                    )
                    nc.any.tensor_copy(xg[:, ns], ps_t[:])
                return xg

            # ---- state tiles ----
            h_sb = cp.tile([B, HIDDEN], BF, tag="h", name="h")
            c_sb = cp.tile([B, HIDDEN], BF, tag="c", name="c")

            # h transpose ring: slot(t) = t % 8, cols (k*8 + slot)*32
            ring = cp.tile([P, NK * 8 * B], BF, tag="ring", name="ring")

            def transpose_h(t):
                """PE-transpose h [32,512] into ring slot t%8 via one psum tile."""
                slot = t % 8
                tp = pt.tile([P, P], BF, tag="tp", name="tp")
                for k in range(NK):
                    nc.tensor.transpose(
                        out=tp[:, k * B : (k + 1) * B],
                        in_=h_sb[:, k * P : (k + 1) * P],
                        identity=identb[:B, :B],
                    )
                dst = ring[:].rearrange("p (k s c) -> p k s c", k=NK, s=8)[
                    :, :, slot, :
                ]
                src = tp[:].rearrange("p (k c) -> p k c", k=NK)
                nc.vector.tensor_copy(dst, src)

            embT_cur = load_embT(0)
            embT_nxt = load_embT(1)

            # ---- initial state h0/c0 = tanh(context @ W + b) in gate tiles ----
            xz0_h = pz.tile([P, HIDDEN], F32, tag="xzg0", name="xz0_h")
            xz0_c = pz.tile([P, HIDDEN], F32, tag="xzg1", name="xz0_c")
            for w_dram, b_sb, dst in (
                (wih_d, bih_sb, xz0_h),
                (wic_d, bic_sb, xz0_c),
            ):
                for kc in range(NKC):
                    wt = sp.tile([P, HIDDEN], BF, tag="wstream", name="wstream")
                    nc.sync.dma_start(out=wt[:], in_=w_dram[kc * P : (kc + 1) * P, :])
                    nc.tensor.matmul(
                        out=dst[:B, :],
                        lhsT=(ctx_sb[:, kc * B : (kc + 1) * B]),
                        rhs=(wt[:]),
                        start=(kc == 0),
                        stop=False,
                    )
                nc.tensor.matmul(
                    out=dst[:B, :],
                    lhsT=(ones1[:1, :B]),
                    rhs=(b_sb[:1, :]),
                    start=False,
                    stop=True,
                )
            nc.scalar.activation(h_sb[:], xz0_h[:B, :], tanh)
            nc.scalar.activation(c_sb[:], xz0_c[:B, :], tanh)
            transpose_h(-1)  # h0 into slot 7

            xg_cur = stage_xg(embT_cur)

            def logits_group(g):
                """Vocab-sharded logits for token tile g from ring slots."""
                half = (g % 2) * 4
                for v in range(NV):
                    vs = slice(v * VS, (v + 1) * VS)
                    pl = pa.tile([P, VS], F32, tag="pl", name="pl")
                    for k in range(NK):
                        cbase = (k * 8 + half) * B
                        nc.tensor.matmul(
                            out=pl[:],
                            lhsT=(ring[:, cbase : cbase + 4 * B]),
                            rhs=(wout_sb[k][:, vs]),
                            start=(k == 0),
                            stop=False,
                        )
                    nc.tensor.matmul(
                        out=pl[:],
                        lhsT=(ones1[:1, :]),
                        rhs=(bout_sb[:1, vs]),
                        start=False,
                        stop=True,
                    )
                    lo = lp.tile([P, VS], BF, tag="lo", name="lo")
                    nc.scalar.copy(lo[:], pl[:])
                    nc.sync.dma_start(out=out_d[g * P : (g + 1) * P, vs], in_=lo[:])

            # ---- main loop ----
            gate_tags = ["xzg0", "xzg1", "xzg2", "xzg3"]
            for g in range(NT):
                # inject staged xg into the per-gate PSUM tiles
                xzg = []
                for n in range(4):
                    zt = pz.tile([P, HIDDEN], F32, tag=gate_tags[n], name=f"xz{n}")
                    nc.tensor.matmul(
                        out=zt[:],
                        lhsT=(identb[:]),
                        rhs=(xg_cur[:, n * HIDDEN : (n + 1) * HIDDEN]),
                        start=True,
                        stop=True,
                    )
                    xzg.append(zt)

                for s in range(4):
                    t = 4 * g + s
                    rows = slice(B * s, B * (s + 1))
                    slot_prev = (t - 1) % 8

                    # recurrent part: z[rows] += h_{t-1} @ W_h (order f, i, g, o)
                    for n in (1, 0, 2, 3):  # Keras gate order is i,f,g,o
                        ns = slice(n * HIDDEN, (n + 1) * HIDDEN)
                        for k in range(NK):
                            cbase = (k * 8 + slot_prev) * B
                            nc.tensor.matmul(
                                out=xzg[n][rows, :],
                                lhsT=(ring[:, cbase : cbase + B]),
                                rhs=(wh_sb[k][:, ns]),
                                start=False,
                                stop=False,
                                tile_position=(0, B * s),
                                skip_group_check=True,
                            )

                    sig_f = gp.tile([B, HIDDEN], BF, tag="sig_f", name="sig_f")
                    sig_i = gp.tile([B, HIDDEN], BF, tag="sig_i", name="sig_i")
                    tanh_g = gp.tile([B, HIDDEN], BF, tag="tanh_g", name="tanh_g")
                    sig_o = gp.tile([B, HIDDEN], BF, tag="sig_o", name="sig_o")
                    nc.scalar.activation(sig_f[:], xzg[1][rows, :], sig)
                    nc.scalar.activation(sig_i[:], xzg[0][rows, :], sig)
                    nc.scalar.activation(tanh_g[:], xzg[2][rows, :], tanh)
                    nc.scalar.activation(sig_o[:], xzg[3][rows, :], sig)

                    t1 = gp.tile([B, HIDDEN], BF, tag="t1", name="t1")
                    t2 = gp.tile([B, HIDDEN], BF, tag="t2", name="t2")
                    c_new = gp.tile([B, HIDDEN], BF, tag="c_new", name="c_new")
                    nc.vector.tensor_mul(t1[:], sig_f[:], c_sb[:])
                    nc.vector.tensor_mul(t2[:], sig_i[:], tanh_g[:])
                    nc.vector.tensor_add(c_new[:], t1[:], t2[:])

                    m_bc = mask_sb[:, t : t + 1].to_broadcast([B, HIDDEN])
                    # masked (token==0) steps carry previous state; in-place blend
                    nc.vector.copy_predicated(c_sb[:], m_bc, c_new[:])

                    # h path uses pre-mask c_new: masked rows discard h_new anyway
                    tanh_c = gp.tile([B, HIDDEN], BF, tag="tanh_c", name="tanh_c")
                    nc.scalar.activation(tanh_c[:], c_new[:], tanh)
                    h_new = gp.tile([B, HIDDEN], BF, tag="h_new", name="h_new")
                    nc.vector.tensor_mul(h_new[:], sig_o[:], tanh_c[:])
                    nc.vector.copy_predicated(h_sb[:], m_bc, h_new[:])

                    transpose_h(t)

                # filler work at very low priority: drips into PE idle gaps
                with low_priority(tc):
                    if g + 1 < NT:
                        xg_cur = stage_xg(embT_nxt)
                        if g + 2 < NT:
                            embT_nxt = load_embT(g + 2)
                    if g >= 1:
                        logits_group(g - 1)

            with low_priority(tc):
                logits_group(NT - 1)

    return nc


def _get_program() -> bass.Bass:
    if "nc" not in _CACHE:
        _CACHE["nc"] = _build_program()
    return _CACHE["nc"]


def prep_in_maps(inputs) -> list:
    import ml_dtypes

    bf16 = ml_dtypes.bfloat16
    tok = np.asarray(inputs["target_tokens"])
    ctx = np.asarray(inputs["context"], dtype=np.float32)
    emb_table = np.asarray(inputs["emb_table"], np.float32)
    w_out = np.asarray(inputs["W_out"], np.float32)
    b_out = np.asarray(inputs["b_out"], np.float32)

    mask = (tok != 0).astype(np.uint8)  # [B, S]
    tok_t = tok.T.reshape(-1).astype(np.int64)  # t*B + b token order
    emb_t = np.ascontiguousarray(emb_table[tok_t].T.astype(bf16))  # [EMBED, T]
    ctx_t = np.ascontiguousarray(ctx.T.astype(bf16))  # [CTX, B]

    shared = {
        "context_t": ctx_t,
        "emb_t": emb_t,
        "w_ih": np.ascontiguousarray(np.asarray(inputs["W_ih"]).astype(bf16)),
        "w_ic": np.ascontiguousarray(np.asarray(inputs["W_ic"]).astype(bf16)),
        "w_x": np.ascontiguousarray(np.asarray(inputs["W_x"]).astype(bf16)),
        "w_h": np.ascontiguousarray(np.asarray(inputs["W_h"]).astype(bf16)),
        "b_g": np.ascontiguousarray(np.asarray(inputs["b"]).astype(bf16)),
        "b_ih": np.ascontiguousarray(np.asarray(inputs["b_ih"]).astype(bf16)),
        "b_ic": np.ascontiguousarray(np.asarray(inputs["b_ic"]).astype(bf16)),
        "maskf": np.ascontiguousarray(mask),
    }
    in_maps = []
    for j in range(NCORES):
        m = dict(shared)
        m["w_out"] = np.ascontiguousarray(w_out[:, j * VSH : (j + 1) * VSH].astype(bf16))
        m["b_out"] = np.ascontiguousarray(b_out[j * VSH : (j + 1) * VSH].astype(bf16))
        in_maps.append(m)
    return in_maps


def kernel(**inputs: np.ndarray) -> np.ndarray:
    in_maps = prep_in_maps(inputs)
    nc = _get_program()
    if not nc.is_finalized():
        nc.finalize()

    import os

    trace = bool(os.environ.get("CAPDEC_TRACE"))
    kw = {}
    if trace:
        kw["trace"] = True
        tdir = os.environ.get("CAPDEC_TRACE_DIR")
        if tdir:
            os.makedirs(tdir, exist_ok=True)
            kw["tmpdir"] = tdir
    bkr = run_bass_kernel_spmd(nc, in_maps, list(range(NCORES)), **kw)
    _CACHE["last_results"] = bkr
    res = bkr.results
    parts = [
        np.asarray(res[j]["logits"]).astype(np.float32).reshape(S, B, VSH)
        for j in range(NCORES)
    ]
    full = np.concatenate(parts, axis=-1)  # [S, B, VOCAB]
    return np.ascontiguousarray(full.transpose(1, 0, 2))
